# revision 53
# baseline (speedup 1.0000x reference)
"""GCN 2-layer + link decode on 8 TRN2 NeuronCores (full inputs in/out).

v3 design (fp16, 4-wide layer-2, local decode):
- Dest-sharded edge parallelism; aggregation commutes with weight matmuls:
  h = relu(segsum(w1*x[src]) @ W1), and the whole decode head collapses to
  y = h @ (W2 @ [WlinA.T|WlinB.T]) per node (4 wide), so layer 2 only
  aggregates 4-wide messages: uv[dst] = segsum(w2 * y[src]).
- fp16 tables/streams; routing by selection-matrix matmuls whose sel is a
  pure batched is_equal (wide DVE ops, stride-0 APs); edge weights folded
  into staged rows by wide in-place mults.
- Layer 1: transposed routing psum_T[c,slot] (no PE transposes downstream);
  consume computes h_T = W1^T agg_T, relu, then y = h @ wu per chunk.
- Only collective: AllGather of the (row-padded) y table, chunked in 4
  row-quarters so it overlaps the layer-1 tail.
- Layer 2: lhsT=sel routing with 4-col rhs; whole window accumulates in one
  PSUM bank; one scalar copy per window evacuates uv.
- Decode fully local (pairs sharded by owner of p0 / p1) and DMA-free:
  pairs grouped into SPMD-uniform uv-chunk cells, routed by PE matmuls
  whose sel matrices are per-partition range indicators (two wide is_ge +
  subtract) reading uv straight from per-window SBUF tiles; decode windows
  are emitted interleaved with layer-2 windows so they pipeline ~one
  window behind the aggregation. Host adds the u and v halves.
"""
import numpy as np

P = 128
N = 100_000
NSHARD = 12_500
SLOTS = 12_544
CHUNKS = SLOTS // P          # 98
TABROWS = 8 * SLOTS          # 100352
RBOUND = [32768, 65536, 98304]
RLO = [0, 32768, 65536, 98304]
NCORES = 8
CALL_CELLS = 16              # chunks per gather-call window
SELBATCH = 16                # sel descs built per wide DVE op
DWIN = 4096                  # decode gather window (pairs)
NWIN = (CHUNKS + CALL_CELLS - 1) // CALL_CELLS


def _range_of(a):
    return np.searchsorted(RBOUND, a, side="right")


def _wrap_idx(a):
    """[NCORES, T] int16 -> [NCORES, 128, T//16] (16-wrap, 8x replicate)."""
    ncr, t = a.shape
    out = a.reshape(ncr, t // 16, 16).transpose(0, 2, 1)
    return np.ascontiguousarray(np.tile(out, (1, 8, 1)))


def _prep_stream(tab_row, slot, w, nchunks, call_cells):
    """SPMD-uniform stream builder (dest-sharded edge streams).

    rel[(p, col)] = within-chunk slot for the desc's entries, -1 otherwise
    (weights live in wst, folded into staged rows, not sel).
    wst[r][c][p, b] = weight of stream entry b*128+p of range r (0 on pads).
    """
    ncr = len(tab_row)
    counts = np.zeros((ncr, nchunks, 4), np.int64)
    for c in range(ncr):
        ch = slot[c] // P
        rr = _range_of(tab_row[c])
        np.add.at(counts, (c, ch, rr), 1)
    estar = counts.max(axis=0)                       # [nchunks, 4]

    layout = []
    for r in range(4):
        calls = []
        base = 0
        for k0 in range(0, nchunks, call_cells):
            k1 = min(k0 + call_cells, nchunks)
            cells = estar[k0:k1, r]
            offs = np.concatenate([[0], np.cumsum(cells)]).astype(np.int64)
            n = int(offs[-1])
            n_pad = max(P, ((n + P - 1) // P) * P)
            calls.append(dict(k0=k0, k1=k1, offs=offs, n=n, n_pad=n_pad,
                              base=base))
            base += n_pad
        layout.append(dict(calls=calls, T=base))

    nwin = (nchunks + call_cells - 1) // call_cells
    wdescs = [[] for _ in range(nwin)]
    selmap = {}
    n_sel = 0
    for wi in range(nwin):
        for r in range(4):
            call = layout[r]["calls"][wi]
            nblk = call["n_pad"] // P
            offs, k0 = call["offs"], call["k0"]
            for b in range(nblk):
                e0, e1 = b * P, b * P + P
                ks = [k for k in range(call["k0"], call["k1"])
                      if offs[k - k0] < e1 and offs[k - k0 + 1] > e0]
                if not ks:
                    ks = [call["k0"]]
                for k in ks:
                    wdescs[wi].append((r, b, k, n_sel))
                    selmap[(r, wi, b, k)] = n_sel
                    n_sel += 1
        covered = set(d[2] for d in wdescs[wi])
        for k in range(wi * call_cells, min((wi + 1) * call_cells, nchunks)):
            if k not in covered:
                wdescs[wi].append((0, 0, k, n_sel))
                n_sel += 1

    idx16 = [np.zeros((ncr, layout[r]["T"]), np.int16) for r in range(4)]
    rel = np.full((ncr, P, n_sel), -1.0, np.float16)
    wst = [np.zeros((ncr, P, layout[r]["T"] // P), np.float16)
           for r in range(4)]

    for c in range(ncr):
        tr, sl, ww = tab_row[c], slot[c], w[c]
        rr = _range_of(tr)
        ch = sl // P
        o = np.lexsort((sl, ch, rr))
        tr, sl, ww, rr, ch = tr[o], sl[o], ww[o], rr[o], ch[o]
        for r in range(4):
            m = rr == r
            if not m.any():
                continue
            trm, slm, wwm, chm = tr[m], sl[m], ww[m], ch[m]
            cell_cnt = np.zeros(nchunks, np.int64)
            np.add.at(cell_cnt, chm, 1)
            cstart = np.concatenate([[0], np.cumsum(cell_cnt)])
            within = np.arange(len(slm)) - cstart[chm]
            wids = chm // call_cells
            calls = layout[r]["calls"]
            cbase = np.array([cl["base"] for cl in calls], np.int64)
            cell_off = np.zeros(nchunks, np.int64)
            for ci, cl in enumerate(calls):
                for k in range(cl["k0"], cl["k1"]):
                    cell_off[k] = cl["offs"][k - cl["k0"]]
            pos = cbase[wids] + cell_off[chm] + within
            idx16[r][c, pos] = (trm - RLO[r]).astype(np.int16)
            wst[r][c][pos % P, pos // P] = wwm.astype(np.float16)
            relpos = pos - cbase[wids]
            blk = relpos // P
            pp = relpos % P
            cols = np.array([selmap[(r, int(w_), int(b_), int(k_))]
                             for w_, b_, k_ in zip(wids, blk, chm)],
                            np.int64)
            rel[c, pp, cols] = (slm % P).astype(np.float16)
    return dict(layout=layout, wdescs=wdescs, n_sel=n_sel, idx16=idx16,
                rel=rel, wst=wst, nwin=nwin)


def kernel(x, edge_index1, edge_index2, edge_weight1, edge_weight2,
           pos_edge_index, W1, W2, Wlin):
    import concourse.bass as bass
    from concourse import bacc, tile, mybir
    from concourse.bass_utils import run_bass_kernel_spmd
    from concourse import library_config

    f32 = mybir.dt.float32
    f16 = mybir.dt.float16
    i16 = mybir.dt.int16
    eq, mu = mybir.AluOpType.is_equal, mybir.AluOpType.mult
    ACT = mybir.ActivationFunctionType

    x = np.asarray(x, np.float32)
    W1 = np.asarray(W1, np.float32)
    W2 = np.asarray(W2, np.float32)
    Wlin = np.asarray(Wlin, np.float32)
    e1 = np.asarray(edge_index1).astype(np.int64)
    e2 = np.asarray(edge_index2).astype(np.int64)
    w1 = np.asarray(edge_weight1, np.float32)
    w2 = np.asarray(edge_weight2, np.float32)
    pe = np.asarray(pos_edge_index).astype(np.int64)

    # ---------- host index preprocessing ----------
    x_tab = np.zeros((TABROWS, P), np.float16)
    x_tab[:N] = x.astype(np.float16)
    n2row = (np.arange(N) // NSHARD) * SLOTS + (np.arange(N) % NSHARD)

    def shard_by_dest(src_rows, dst, w):
        owner = dst // NSHARD
        ld = dst - owner * NSHARD
        return ([src_rows[owner == c] for c in range(NCORES)],
                [ld[owner == c] for c in range(NCORES)],
                [w[owner == c] for c in range(NCORES)])

    l1 = _prep_stream(*shard_by_dest(e1[0], e1[1], w1), CHUNKS, CALL_CELLS)

    # y_tab row order: (window, core, slot-in-window) so each window's
    # AllGather output is one contiguous block.
    WROWS = CALL_CELLS * P
    wrows = [min((wi + 1) * CALL_CELLS, CHUNKS) * P - wi * CALL_CELLS * P
             for wi in range(NWIN)]
    wbase8 = np.concatenate([[0], np.cumsum(np.array(wrows) * NCORES)])

    def n2row_y(idx):
        c = idx // NSHARD
        s = idx % NSHARD
        w = np.minimum(s // WROWS, NWIN - 1)
        return (wbase8[w] + c * np.array(wrows)[w] + s - w * WROWS)

    l2 = _prep_stream(*shard_by_dest(n2row_y(e2[0]), e2[1], w2),
                      CHUNKS, CALL_CELLS)

    npairs = pe.shape[1]
    own0 = pe[0] // NSHARD
    own1 = pe[1] // NSHARD
    zrow0 = pe[0] % NSHARD
    zrow1 = pe[1] % NSHARD
    u_idx = [zrow0[own0 == c] for c in range(NCORES)]
    v_idx = [zrow1[own1 == c] for c in range(NCORES)]

    def prep_decode(zlists):
        """PE-routed decode: pairs grouped by uv-chunk (cells max-padded
        across cores), sel = per-partition range indicator.

        Returns descs [(blk, k)], dfirst/dend [ncr, P, ndesc] f16, and
        per-core stream position of every pair (for host unshuffle)."""
        cnts = np.zeros((NCORES, CHUNKS), np.int64)
        for c in range(NCORES):
            np.add.at(cnts, (c, zlists[c] // P), 1)
        nk = cnts.max(axis=0)
        offs = np.concatenate([[0], np.cumsum(nk)]).astype(np.int64)
        T = ((int(offs[-1]) + P - 1) // P) * P
        nblk = T // P
        descs = []
        for b in range(nblk):
            e0, e1 = b * P, b * P + P
            ks = [k for k in range(CHUNKS)
                  if offs[k] < e1 and offs[k + 1] > e0]
            if not ks:
                ks = [0]
            for k in ks:
                descs.append((b, k))
        ndesc = len(descs)
        dfirst = np.zeros((NCORES, P, ndesc), np.float16)
        dend = np.zeros((NCORES, P, ndesc), np.float16)
        pos_all = []
        for c in range(NCORES):
            z = zlists[c]
            k = z // P
            order = np.lexsort((np.arange(len(z)), z))
            rank = np.empty(len(z), np.int64)
            csort = np.concatenate([[0], np.cumsum(
                np.bincount(k, minlength=CHUNKS))])
            rank[order] = np.arange(len(z)) - csort[k[order]]
            pos = offs[k] + rank
            pos_all.append(pos)
            zs = np.sort(z)
            kcnt = np.bincount(k, minlength=CHUNKS)
            kcs = np.concatenate([[0], np.cumsum(kcnt)])
            for j, (b, kk) in enumerate(descs):
                zcell = zs[kcs[kk]:kcs[kk + 1]] - kk * P
                if len(zcell) == 0:
                    continue
                g0 = np.searchsorted(zcell, np.arange(P)) + offs[kk]
                g1 = np.searchsorted(zcell, np.arange(P), "right") + offs[kk]
                f = np.clip(g0, b * P, b * P + P) - b * P
                e = np.clip(g1, b * P, b * P + P) - b * P
                dfirst[c, :, j] = f.astype(np.float16)
                dend[c, :, j] = e.astype(np.float16)
        return dict(descs=descs, ndesc=ndesc, nblk=nblk,
                    dfirst=dfirst, dend=dend, pos=pos_all, T=T)

    du = prep_decode(u_idx)
    dv = prep_decode(v_idx)

    idx_arr = {}
    for key, pr in (("l1", l1), ("l2", l2)):
        for r in range(4):
            idx_arr[(key, r)] = _wrap_idx(pr["idx16"][r])

    # wu = W2 @ [WlinA.T | WlinB.T]  [128, 4]
    wcat = np.concatenate([Wlin[:, :P].T, Wlin[:, P:].T], axis=1)
    wu = (W2 @ wcat).astype(np.float16)

    iota_np = np.broadcast_to(
        np.arange(P, dtype=np.float16)[None, :], (P, P)).copy()

    # ---------- device program ----------
    nc = bacc.Bacc("TRN2", target_bir_lowering=False, debug=False,
                   num_devices=NCORES, num_swdge_queues=4)

    def din(name, shape, dt=f16):
        return nc.dram_tensor(name, list(shape), dt, kind="ExternalInput").ap()

    xt = din("x_tab", (TABROWS, P))
    w1t = din("W1r", (P, P))
    wu_in = din("wu4", (P, 4))
    iota_in = din("iota", (P, P))
    idx_in = {k: din(f"idx_{k[0]}_{k[1]}", v.shape[1:], i16)
              for k, v in idx_arr.items()}
    rel_in = {key: din(f"rel_{key}", (P, pr["n_sel"]))
              for key, pr in (("l1", l1), ("l2", l2))}
    wst_in = {(key, r): din(f"wst_{key}_{r}", (P, pr["layout"][r]["T"] // P))
              for key, pr in (("l1", l1), ("l2", l2)) for r in range(4)}
    dfe_in = {("u", 0): din("du_first", (P, du["ndesc"])),
              ("u", 1): din("du_end", (P, du["ndesc"])),
              ("v", 0): din("dv_first", (P, dv["ndesc"])),
              ("v", 1): din("dv_end", (P, dv["ndesc"]))}

    u_out = nc.dram_tensor("u_out", [P, du["nblk"] * 4], f16,
                           kind="ExternalOutput").ap()
    v_out = nc.dram_tensor("v_out", [P, dv["nblk"] * 4], f16,
                           kind="ExternalOutput").ap()
    y_w = [nc.dram_tensor(f"y_w{q}", [wrows[q], P], f16).ap()
           for q in range(NWIN)]
    # y table split per int16-gather range so layer-2 range-r gathers only
    # depend on that range's AllGather parts (AG windows align 2:1 with
    # ranges when CALL_CELLS=16: 8*2048=16384 rows per part).
    _rsize = [RBOUND[0], RBOUND[1] - RBOUND[0], RBOUND[2] - RBOUND[1],
              TABROWS - RBOUND[2]]
    y_tr = [nc.dram_tensor(f"y_tr{r}", [_rsize[r], P], f16,
                           addr_space="Shared") for r in range(4)]

    qn = [0]

    def next_q():
        qn[0] = (qn[0] + 1) % 4
        return qn[0]

    with tile.TileContext(nc) as tc:
        with (
            tc.tile_pool(name="meta", bufs=1) as mp,
            tc.tile_pool(name="stage", bufs=3) as sgp,
            tc.tile_pool(name="idxp", bufs=1) as ixp,
            tc.tile_pool(name="selp", bufs=4) as selp,
            tc.tile_pool(name="work", bufs=4) as wp,
            tc.tile_pool(name="ypad", bufs=2) as ypp,
            tc.tile_pool(name="dago", bufs=2) as dgp,
            tc.tile_pool(name="psA", bufs=6, space="PSUM") as ppA,
            tc.tile_pool(name="psB", bufs=2, space="PSUM") as ppB,
        ):
            nc.gpsimd.load_library(library_config.mlp)
            iota_t = mp.tile([P, P], f16, name="iota_t")
            nc.sync.dma_start(iota_t[:], iota_in[:])
            w1_sb = mp.tile([P, P], f16, name="w1_sb")
            nc.sync.dma_start(w1_sb[:], w1t[:])
            wu_sb = mp.tile([P, 4], f16, name="wu_sb")
            nc.sync.dma_start(wu_sb[:], wu_in[:])
            # per-l2-window uv tiles so decode can start before l2 finishes
            uv_w = [mp.tile([P, 4 * CALL_CELLS], f16, name=f"uv_w{wi}")
                    for wi in range(NWIN)]

            def run_layer(key, pr, tabs, mode, after_window=None):
                rel_sb = ixp.tile([P, pr["n_sel"]], f16,
                                  name=f"rel_{key}", tag="relt")
                nc.sync.dma_start(rel_sb[:], rel_in[key][:])
                idx_sb = []
                wst_sb = []
                for r in range(4):
                    cols = pr["layout"][r]["T"] // 16
                    it = ixp.tile([P, cols], i16, name=f"ix_{key}_{r}",
                                  tag=f"ix{r}")
                    nc.sync.dma_start(it[:], idx_in[(key, r)][:])
                    idx_sb.append(it)
                    wt = ixp.tile([P, pr["layout"][r]["T"] // P], f16,
                                  name=f"wst_{key}_{r}", tag=f"wst{r}")
                    nc.sync.dma_start(wt[:], wst_in[(key, r)][:])
                    wst_sb.append(wt)

                for wi in range(pr["nwin"]):
                    k0 = wi * CALL_CELLS
                    k1 = min(k0 + CALL_CELLS, CHUNKS)
                    nk = k1 - k0
                    st_t = {}
                    for r in range(4):
                        call = pr["layout"][r]["calls"][wi]
                        npad = call["n_pad"]
                        nblk = npad // P
                        st = sgp.tile([P, nblk * P], f16,
                                      name=f"st_{key}_{wi}_{r}",
                                      tag=f"stage{r}")
                        nc.gpsimd.dma_gather(
                            st[:].rearrange("p (c e) -> p c e", e=P),
                            tabs[r],
                            idx_sb[r][:, call["base"] // 16:
                                      (call["base"] + npad) // 16],
                            npad, npad, P,
                            queue_num=next_q(), single_packet=False)
                        b0 = call["base"] // P
                        if mode == "T":
                            w_exp = (wst_sb[r][:, b0:b0 + nblk].unsqueeze(2)
                                     .to_broadcast([P, nblk, P]))
                            nc.vector.tensor_tensor(
                                out=st[:].rearrange("p (b c) -> p b c", c=P),
                                in0=st[:].rearrange("p (b c) -> p b c", c=P),
                                in1=w_exp, op=mu)
                        else:
                            w_exp = (wst_sb[r][:, b0:b0 + nblk].unsqueeze(2)
                                     .to_broadcast([P, nblk, 4]))
                            v4 = st[:].rearrange(
                                "p (b c) -> p b c", c=P)[:, :, 0:4]
                            nc.vector.tensor_tensor(
                                out=v4, in0=v4, in1=w_exp, op=mu)
                        st_t[r] = st

                    descs = pr["wdescs"][wi]
                    if mode == "T":
                        nbank = (CALL_CELLS + 3) // 4
                        banks = [ppA.tile([P, 4 * P], f32, space="PSUM",
                                          name=f"pt_{key}_{wi}_{t}",
                                          tag="agg")
                                 for t in range(nbank)]

                        def pslice(k):
                            d = k - k0
                            return banks[d // 4][:, (d % 4) * P:
                                                 (d % 4) * P + P]

                        def bank_of(k):
                            return (k - k0) // 4
                    else:
                        wtile = ppA.tile([P, 4 * CALL_CELLS], f32,
                                         space="PSUM",
                                         name=f"pt_{key}_{wi}", tag="agg")

                        def pslice(k):
                            d = k - k0
                            return wtile[:, d * 4:d * 4 + 4]

                        def bank_of(k):
                            return 0

                    first = {}
                    last = {}
                    for j, (r, b, k, s) in enumerate(descs):
                        bk = bank_of(k)
                        first.setdefault(bk, j)
                        last[bk] = j
                    sel_t = {}
                    for j0 in range(0, len(descs), SELBATCH):
                        nb = min(SELBATCH, len(descs) - j0)
                        s0 = descs[j0][3]
                        selt = selp.tile([P, nb * P], f16,
                                         name=f"sel_{key}_{wi}_{j0}",
                                         tag="sel")
                        nc.vector.tensor_tensor(
                            out=selt[:].rearrange("p (b c) -> p b c", c=P),
                            in0=iota_t[:].unsqueeze(1).to_broadcast(
                                [P, nb, P]),
                            in1=rel_sb[:, s0:s0 + nb].unsqueeze(2)
                                .to_broadcast([P, nb, P]),
                            op=eq)
                        for jj in range(nb):
                            sel_t[j0 + jj] = (selt, jj)
                    for j, (r, b, k, s) in enumerate(descs):
                        selt, jj = sel_t[j]
                        bk = bank_of(k)
                        if mode == "T":
                            nc.tensor.matmul(
                                pslice(k),
                                lhsT=st_t[r][:, b * P:(b + 1) * P],
                                rhs=selt[:, jj * P:(jj + 1) * P],
                                start=(first[bk] == j), stop=(last[bk] == j),
                                skip_group_check=True)
                        else:
                            nc.tensor.matmul(
                                pslice(k),
                                lhsT=selt[:, jj * P:(jj + 1) * P],
                                rhs=st_t[r][:, b * P:b * P + 4],
                                start=(first[bk] == j), stop=(last[bk] == j),
                                skip_group_check=True)

                    if mode == "T":
                        # consume: h_T = W1^T agg_T, relu, y = h @ wu
                        yp = ypp.tile([P, nk * P], f16,
                                      name=f"yp_{wi}", tag="yp")
                        nc.vector.memset(yp[:], 0)
                        for k in range(k0, k1):
                            g = k - k0
                            at_sb = wp.tile([P, P], f16, name=f"a1_{k}",
                                            tag="at")
                            nc.scalar.activation(at_sb[:], pslice(k),
                                                 ACT.Copy)
                            ht_ps = ppB.tile([P, P], f32, space="PSUM",
                                             name=f"ht_{k}", tag="psB")
                            nc.tensor.matmul(ht_ps[:], lhsT=w1_sb[:],
                                             rhs=at_sb[:],
                                             start=True, stop=True)
                            ht_sb = wp.tile([P, P], f16, name=f"hts_{k}",
                                            tag="ht")
                            nc.scalar.activation(ht_sb[:], ht_ps[:],
                                                 ACT.Relu)
                            y_ps = ppB.tile([P, 4], f32, space="PSUM",
                                            name=f"y_{k}", tag="psB")
                            nc.tensor.matmul(y_ps[:], lhsT=ht_sb[:],
                                             rhs=wu_sb[:],
                                             start=True, stop=True)
                            nc.scalar.activation(
                                yp[:].rearrange("p (g c) -> p g c",
                                                c=P)[:, g:g + 1, 0:4],
                                y_ps[:].unsqueeze(1), ACT.Copy)
                        nc.sync.dma_start(
                            y_w[wi][:].rearrange("(g p) f -> p g f", p=P),
                            yp[:].rearrange("p (g c) -> p g c", c=P))
                    else:
                        nc.scalar.activation(
                            uv_w[wi][:, 0:nk * 4],
                            wtile[:, 0:nk * 4], ACT.Copy)
                    if after_window is not None:
                        after_window(wi)

            run_layer("l1", l1, [xt[RLO[r]:] for r in range(4)], "T")
            for q in range(NWIN):
                g0 = int(wbase8[q])
                g1 = int(wbase8[q + 1])
                r = int(_range_of(np.array([g0]))[0])
                off = g0 - RLO[r]
                assert g1 <= RLO[r] + _rsize[r], (q, g0, g1, r)
                nc.gpsimd.collective_compute(
                    "AllGather", mybir.AluOpType.bypass,
                    replica_groups=[list(range(NCORES))],
                    ins=[y_w[q][:]],
                    outs=[y_tr[r].ap()[off:off + (g1 - g0), :]])
            # ---------- decode: PE routing out of uv_w, interleaved with l2
            DW = 24  # pair-chunks per psum bank window
            dload = {}
            for name, pr in (("u", du), ("v", dv)):
                fsb = ixp.tile([P, pr["ndesc"]], f16,
                               name=f"df_{name}", tag=f"df_{name}")
                nc.sync.dma_start(fsb[:], dfe_in[(name, 0)][:])
                esb = ixp.tile([P, pr["ndesc"]], f16,
                               name=f"de_{name}", tag=f"de_{name}")
                nc.sync.dma_start(esb[:], dfe_in[(name, 1)][:])
                dload[name] = (fsb, esb)

            def emit_decode_window(name, pr, b0):
                outd = u_out if name == "u" else v_out
                fsb, esb = dload[name]
                descs = pr["descs"]
                b1 = min(b0 + DW, pr["nblk"])
                wd = [(j, d) for j, d in enumerate(descs)
                      if b0 <= d[0] < b1]
                bank = ppA.tile([P, 4 * DW], f32, space="PSUM",
                                name=f"pd_{name}_{b0}", tag="agg")
                sel_t = {}
                for i0 in range(0, len(wd), SELBATCH):
                    nb = min(SELBATCH, len(wd) - i0)
                    s0 = wd[i0][0]
                    t1 = dgp.tile([P, nb * P], f16,
                                  name=f"t1_{name}_{b0}_{i0}", tag="t1")
                    t2 = dgp.tile([P, nb * P], f16,
                                  name=f"t2_{name}_{b0}_{i0}", tag="t2")
                    selt = selp.tile([P, nb * P], f16,
                                     name=f"sd_{name}_{b0}_{i0}", tag="sel")
                    io_rep = iota_t[:].unsqueeze(1).to_broadcast([P, nb, P])
                    nc.vector.tensor_tensor(
                        out=t1[:].rearrange("p (b c) -> p b c", c=P),
                        in0=io_rep,
                        in1=fsb[:, s0:s0 + nb].unsqueeze(2)
                            .to_broadcast([P, nb, P]),
                        op=mybir.AluOpType.is_ge)
                    nc.vector.tensor_tensor(
                        out=t2[:].rearrange("p (b c) -> p b c", c=P),
                        in0=io_rep,
                        in1=esb[:, s0:s0 + nb].unsqueeze(2)
                            .to_broadcast([P, nb, P]),
                        op=mybir.AluOpType.is_ge)
                    nc.vector.tensor_tensor(
                        out=selt[:], in0=t1[:], in1=t2[:],
                        op=mybir.AluOpType.subtract)
                    for jj in range(nb):
                        sel_t[i0 + jj] = (selt, jj)
                first = {}
                last = {}
                for i, (j, (b, k)) in enumerate(wd):
                    first.setdefault(0, i)
                    last[0] = i
                for i, (j, (b, k)) in enumerate(wd):
                    selt, jj = sel_t[i]
                    nc.tensor.matmul(
                        bank[:, (b - b0) * 4:(b - b0) * 4 + 4],
                        lhsT=selt[:, jj * P:(jj + 1) * P],
                        rhs=uv_w[k // CALL_CELLS][
                            :, 4 * (k % CALL_CELLS):4 * (k % CALL_CELLS) + 4],
                        start=(first[0] == i), stop=(last[0] == i),
                        skip_group_check=True)
                dsb = dgp.tile([P, (b1 - b0) * 4], f16,
                               name=f"ds_{name}_{b0}", tag="ds")
                nc.scalar.activation(dsb[:], bank[:, 0:(b1 - b0) * 4],
                                     ACT.Copy)
                nc.sync.dma_start(outd[:, b0 * 4:b1 * 4], dsb[:])

            # decode window -> highest uv-chunk it needs
            dmaxk = {}
            for name, pr in (("u", du), ("v", dv)):
                for b0 in range(0, pr["nblk"], DW):
                    b1 = min(b0 + DW, pr["nblk"])
                    dmaxk[(name, b0)] = max(
                        k for (b, k) in pr["descs"] if b0 <= b < b1)
            dqueue = sorted(dmaxk.items(), key=lambda kv: kv[1])
            dpos = [0]

            def pump_decode(wi):
                kdone = min((wi + 1) * CALL_CELLS, CHUNKS) - 1
                while (dpos[0] < len(dqueue)
                       and dqueue[dpos[0]][1] <= kdone):
                    (nm, b0), _ = dqueue[dpos[0]]
                    emit_decode_window(nm, du if nm == "u" else dv, b0)
                    dpos[0] += 1

            run_layer("l2", l2, [y_tr[r].ap() for r in range(4)], "4",
                      after_window=pump_decode)
            while dpos[0] < len(dqueue):
                (nm, b0), _ = dqueue[dpos[0]]
                emit_decode_window(nm, du if nm == "u" else dv, b0)
                dpos[0] += 1

    nc.compile()

    # ---------- stage inputs & run ----------
    in_maps = []
    for c in range(NCORES):
        m = {"x_tab": x_tab, "W1r": W1.astype(np.float16),
             "wu4": wu, "iota": iota_np,
             "du_first": np.ascontiguousarray(du["dfirst"][c]),
             "du_end": np.ascontiguousarray(du["dend"][c]),
             "dv_first": np.ascontiguousarray(dv["dfirst"][c]),
             "dv_end": np.ascontiguousarray(dv["dend"][c])}
        for key, pr in (("l1", l1), ("l2", l2)):
            m[f"rel_{key}"] = np.ascontiguousarray(pr["rel"][c])
            for r in range(4):
                m[f"idx_{key}_{r}"] = idx_arr[(key, r)][c]
                m[f"wst_{key}_{r}"] = np.ascontiguousarray(pr["wst"][r][c])
        in_maps.append(m)

    res = run_bass_kernel_spmd(nc, in_maps, core_ids=list(range(NCORES)),
                               trace=globals().get("TRACE", False))
    globals()["LAST_EXEC_NS"] = res.exec_time_ns
    globals()["LAST_RES"] = res

    def unpack(key, pr):
        a = np.stack([res.results[c][key] for c in range(NCORES)])
        a = a.reshape(NCORES, P, pr["nblk"], 4).transpose(0, 2, 1, 3)
        return a.reshape(NCORES, pr["T"], 4).astype(np.float32)

    u_res = unpack("u_out", du)
    v_res = unpack("v_out", dv)
    # stream position of each pair within its core's decode stream
    pos0 = np.zeros(npairs, np.int64)
    pos1 = np.zeros(npairs, np.int64)
    for c in range(NCORES):
        pos0[own0 == c] = du["pos"][c]
        pos1[own1 == c] = dv["pos"][c]
    out = u_res[own0, pos0, 0:2] + v_res[own1, pos1, 2:4]
    return np.ascontiguousarray(out, np.float32)


# revision 55
# speedup vs baseline: 1.1239x; 1.1239x over previous
"""GCN 2-layer + link decode on 8 TRN2 NeuronCores (full inputs in/out).

v3 design (fp16, 4-wide layer-2, local decode):
- Dest-sharded edge parallelism; aggregation commutes with weight matmuls:
  h = relu(segsum(w1*x[src]) @ W1), and the whole decode head collapses to
  y = h @ (W2 @ [WlinA.T|WlinB.T]) per node (4 wide), so layer 2 only
  aggregates 4-wide messages: uv[dst] = segsum(w2 * y[src]).
- fp16 tables/streams; routing by selection-matrix matmuls whose sel is a
  pure batched is_equal (wide DVE ops, stride-0 APs); edge weights folded
  into staged rows by wide in-place mults.
- Layer 1: transposed routing psum_T[c,slot] (no PE transposes downstream);
  consume computes h_T = W1^T agg_T, relu, then y = h @ wu per chunk.
- Only collective: AllGather of the (row-padded) y table, chunked in 4
  row-quarters so it overlaps the layer-1 tail.
- Layer 2: lhsT=sel routing with 4-col rhs; whole window accumulates in one
  PSUM bank; one scalar copy per window evacuates uv.
- Decode fully local (pairs sharded by owner of p0 / p1) and DMA-free:
  pairs grouped into SPMD-uniform uv-chunk cells, routed by PE matmuls
  whose sel matrices are per-partition range indicators (two wide is_ge +
  subtract) reading uv straight from per-window SBUF tiles; decode windows
  are emitted interleaved with layer-2 windows so they pipeline ~one
  window behind the aggregation. Host adds the u and v halves.
"""
import numpy as np

P = 128
N = 100_000
NSHARD = 12_500
SLOTS = 12_544
CHUNKS = SLOTS // P          # 98
TABROWS = 8 * SLOTS          # 100352
RBOUND = [32768, 65536, 98304]
RLO = [0, 32768, 65536, 98304]
NCORES = 8
CALL_CELLS = 16              # chunks per gather-call window
SELBATCH = 16                # sel descs built per wide DVE op
DWIN = 4096                  # decode gather window (pairs)
NWIN = (CHUNKS + CALL_CELLS - 1) // CALL_CELLS


def _range_of(a):
    return np.searchsorted(RBOUND, a, side="right")


def _wrap_idx(a):
    """[NCORES, T] int16 -> [NCORES, 128, T//16] (16-wrap, 8x replicate)."""
    ncr, t = a.shape
    out = a.reshape(ncr, t // 16, 16).transpose(0, 2, 1)
    return np.ascontiguousarray(np.tile(out, (1, 8, 1)))


def _prep_stream(tab_row, slot, w, nchunks, call_cells):
    """SPMD-uniform stream builder (dest-sharded edge streams).

    rel[(p, col)] = within-chunk slot for the desc's entries, -1 otherwise
    (weights live in wst, folded into staged rows, not sel).
    wst[r][c][p, b] = weight of stream entry b*128+p of range r (0 on pads).
    """
    ncr = len(tab_row)
    counts = np.zeros((ncr, nchunks, 4), np.int64)
    for c in range(ncr):
        ch = slot[c] // P
        rr = _range_of(tab_row[c])
        np.add.at(counts, (c, ch, rr), 1)
    estar = counts.max(axis=0)                       # [nchunks, 4]

    layout = []
    for r in range(4):
        calls = []
        base = 0
        for k0 in range(0, nchunks, call_cells):
            k1 = min(k0 + call_cells, nchunks)
            cells = estar[k0:k1, r]
            offs = np.concatenate([[0], np.cumsum(cells)]).astype(np.int64)
            n = int(offs[-1])
            n_pad = max(P, ((n + P - 1) // P) * P)
            calls.append(dict(k0=k0, k1=k1, offs=offs, n=n, n_pad=n_pad,
                              base=base))
            base += n_pad
        layout.append(dict(calls=calls, T=base))

    nwin = (nchunks + call_cells - 1) // call_cells
    wdescs = [[] for _ in range(nwin)]
    selmap = {}
    n_sel = 0
    for wi in range(nwin):
        for r in range(4):
            call = layout[r]["calls"][wi]
            nblk = call["n_pad"] // P
            offs, k0 = call["offs"], call["k0"]
            for b in range(nblk):
                e0, e1 = b * P, b * P + P
                ks = [k for k in range(call["k0"], call["k1"])
                      if offs[k - k0] < e1 and offs[k - k0 + 1] > e0]
                if not ks:
                    ks = [call["k0"]]
                for k in ks:
                    wdescs[wi].append((r, b, k, n_sel))
                    selmap[(r, wi, b, k)] = n_sel
                    n_sel += 1
        covered = set(d[2] for d in wdescs[wi])
        for k in range(wi * call_cells, min((wi + 1) * call_cells, nchunks)):
            if k not in covered:
                wdescs[wi].append((0, 0, k, n_sel))
                n_sel += 1

    idx16 = [np.zeros((ncr, layout[r]["T"]), np.int16) for r in range(4)]
    rel = np.full((ncr, P, n_sel), -1.0, np.float16)
    wst = [np.zeros((ncr, P, layout[r]["T"] // P), np.float16)
           for r in range(4)]

    for c in range(ncr):
        tr, sl, ww = tab_row[c], slot[c], w[c]
        rr = _range_of(tr)
        ch = sl // P
        o = np.lexsort((sl, ch, rr))
        tr, sl, ww, rr, ch = tr[o], sl[o], ww[o], rr[o], ch[o]
        for r in range(4):
            m = rr == r
            if not m.any():
                continue
            trm, slm, wwm, chm = tr[m], sl[m], ww[m], ch[m]
            cell_cnt = np.zeros(nchunks, np.int64)
            np.add.at(cell_cnt, chm, 1)
            cstart = np.concatenate([[0], np.cumsum(cell_cnt)])
            within = np.arange(len(slm)) - cstart[chm]
            wids = chm // call_cells
            calls = layout[r]["calls"]
            cbase = np.array([cl["base"] for cl in calls], np.int64)
            cell_off = np.zeros(nchunks, np.int64)
            for ci, cl in enumerate(calls):
                for k in range(cl["k0"], cl["k1"]):
                    cell_off[k] = cl["offs"][k - cl["k0"]]
            pos = cbase[wids] + cell_off[chm] + within
            idx16[r][c, pos] = (trm - RLO[r]).astype(np.int16)
            wst[r][c][pos % P, pos // P] = wwm.astype(np.float16)
            relpos = pos - cbase[wids]
            blk = relpos // P
            pp = relpos % P
            cols = np.array([selmap[(r, int(w_), int(b_), int(k_))]
                             for w_, b_, k_ in zip(wids, blk, chm)],
                            np.int64)
            rel[c, pp, cols] = (slm % P).astype(np.float16)
    return dict(layout=layout, wdescs=wdescs, n_sel=n_sel, idx16=idx16,
                rel=rel, wst=wst, nwin=nwin)


def kernel(x, edge_index1, edge_index2, edge_weight1, edge_weight2,
           pos_edge_index, W1, W2, Wlin):
    import concourse.bass as bass
    from concourse import bacc, tile, mybir
    from concourse.bass_utils import run_bass_kernel_spmd
    from concourse import library_config

    f32 = mybir.dt.float32
    f16 = mybir.dt.float16
    i16 = mybir.dt.int16
    eq, mu = mybir.AluOpType.is_equal, mybir.AluOpType.mult
    ACT = mybir.ActivationFunctionType

    x = np.asarray(x, np.float32)
    W1 = np.asarray(W1, np.float32)
    W2 = np.asarray(W2, np.float32)
    Wlin = np.asarray(Wlin, np.float32)
    e1 = np.asarray(edge_index1).astype(np.int64)
    e2 = np.asarray(edge_index2).astype(np.int64)
    w1 = np.asarray(edge_weight1, np.float32)
    w2 = np.asarray(edge_weight2, np.float32)
    pe = np.asarray(pos_edge_index).astype(np.int64)

    # ---------- host index preprocessing ----------
    x_tab = np.zeros((TABROWS, P), np.float16)
    x_tab[:N] = x.astype(np.float16)
    n2row = (np.arange(N) // NSHARD) * SLOTS + (np.arange(N) % NSHARD)

    def shard_by_dest(src_rows, dst, w):
        owner = dst // NSHARD
        ld = dst - owner * NSHARD
        return ([src_rows[owner == c] for c in range(NCORES)],
                [ld[owner == c] for c in range(NCORES)],
                [w[owner == c] for c in range(NCORES)])

    l1 = _prep_stream(*shard_by_dest(e1[0], e1[1], w1), CHUNKS, CALL_CELLS)

    # y_tab row order: (window, core, slot-in-window) so each window's
    # AllGather output is one contiguous block.
    WROWS = CALL_CELLS * P
    wrows = [min((wi + 1) * CALL_CELLS, CHUNKS) * P - wi * CALL_CELLS * P
             for wi in range(NWIN)]
    wbase8 = np.concatenate([[0], np.cumsum(np.array(wrows) * NCORES)])

    def n2row_y(idx):
        c = idx // NSHARD
        s = idx % NSHARD
        w = np.minimum(s // WROWS, NWIN - 1)
        return (wbase8[w] + c * np.array(wrows)[w] + s - w * WROWS)

    l2 = _prep_stream(*shard_by_dest(n2row_y(e2[0]), e2[1], w2),
                      CHUNKS, CALL_CELLS)

    npairs = pe.shape[1]
    own0 = pe[0] // NSHARD
    own1 = pe[1] // NSHARD
    zrow0 = pe[0] % NSHARD
    zrow1 = pe[1] % NSHARD
    u_idx = [zrow0[own0 == c] for c in range(NCORES)]
    v_idx = [zrow1[own1 == c] for c in range(NCORES)]

    def prep_decode(zlists):
        """PE-routed decode: pairs grouped by uv-chunk (cells max-padded
        across cores), sel = per-partition range indicator.

        Returns descs [(blk, k)], dfirst/dend [ncr, P, ndesc] f16, and
        per-core stream position of every pair (for host unshuffle)."""
        cnts = np.zeros((NCORES, CHUNKS), np.int64)
        for c in range(NCORES):
            np.add.at(cnts, (c, zlists[c] // P), 1)
        nk = cnts.max(axis=0)
        offs = np.concatenate([[0], np.cumsum(nk)]).astype(np.int64)
        T = ((int(offs[-1]) + P - 1) // P) * P
        nblk = T // P
        descs = []
        for b in range(nblk):
            e0, e1 = b * P, b * P + P
            ks = [k for k in range(CHUNKS)
                  if offs[k] < e1 and offs[k + 1] > e0]
            if not ks:
                ks = [0]
            for k in ks:
                descs.append((b, k))
        ndesc = len(descs)
        dfirst = np.zeros((NCORES, P, ndesc), np.float16)
        dend = np.zeros((NCORES, P, ndesc), np.float16)
        pos_all = []
        for c in range(NCORES):
            z = zlists[c]
            k = z // P
            order = np.lexsort((np.arange(len(z)), z))
            rank = np.empty(len(z), np.int64)
            csort = np.concatenate([[0], np.cumsum(
                np.bincount(k, minlength=CHUNKS))])
            rank[order] = np.arange(len(z)) - csort[k[order]]
            pos = offs[k] + rank
            pos_all.append(pos)
            zs = np.sort(z)
            kcnt = np.bincount(k, minlength=CHUNKS)
            kcs = np.concatenate([[0], np.cumsum(kcnt)])
            for j, (b, kk) in enumerate(descs):
                zcell = zs[kcs[kk]:kcs[kk + 1]] - kk * P
                if len(zcell) == 0:
                    continue
                g0 = np.searchsorted(zcell, np.arange(P)) + offs[kk]
                g1 = np.searchsorted(zcell, np.arange(P), "right") + offs[kk]
                f = np.clip(g0, b * P, b * P + P) - b * P
                e = np.clip(g1, b * P, b * P + P) - b * P
                dfirst[c, :, j] = f.astype(np.float16)
                dend[c, :, j] = e.astype(np.float16)
        return dict(descs=descs, ndesc=ndesc, nblk=nblk,
                    dfirst=dfirst, dend=dend, pos=pos_all, T=T)

    du = prep_decode(u_idx)
    dv = prep_decode(v_idx)

    idx_arr = {}
    for key, pr in (("l1", l1), ("l2", l2)):
        for r in range(4):
            idx_arr[(key, r)] = _wrap_idx(pr["idx16"][r])

    # wu = W2 @ [WlinA.T | WlinB.T]  [128, 4]
    wcat = np.concatenate([Wlin[:, :P].T, Wlin[:, P:].T], axis=1)
    wu = (W2 @ wcat).astype(np.float16)

    iota_np = np.broadcast_to(
        np.arange(P, dtype=np.float16)[None, :], (P, P)).copy()

    # ---------- device program ----------
    nc = bacc.Bacc("TRN2", target_bir_lowering=False, debug=False,
                   num_devices=NCORES, num_swdge_queues=4)

    def din(name, shape, dt=f16):
        return nc.dram_tensor(name, list(shape), dt, kind="ExternalInput").ap()

    xt = din("x_tab", (TABROWS, P))
    w1t = din("W1r", (P, P))
    wu_in = din("wu4", (P, 4))
    iota_in = din("iota", (P, P))
    idx_in = {k: din(f"idx_{k[0]}_{k[1]}", v.shape[1:], i16)
              for k, v in idx_arr.items()}
    rel_in = {key: din(f"rel_{key}", (P, pr["n_sel"]))
              for key, pr in (("l1", l1), ("l2", l2))}
    wst_in = {(key, r): din(f"wst_{key}_{r}", (P, pr["layout"][r]["T"] // P))
              for key, pr in (("l1", l1), ("l2", l2)) for r in range(4)}
    dfe_in = {("u", 0): din("du_first", (P, du["ndesc"])),
              ("u", 1): din("du_end", (P, du["ndesc"])),
              ("v", 0): din("dv_first", (P, dv["ndesc"])),
              ("v", 1): din("dv_end", (P, dv["ndesc"]))}

    u_out = nc.dram_tensor("u_out", [P, du["nblk"] * 4], f16,
                           kind="ExternalOutput").ap()
    v_out = nc.dram_tensor("v_out", [P, dv["nblk"] * 4], f16,
                           kind="ExternalOutput").ap()
    y_w = [nc.dram_tensor(f"y_w{q}", [wrows[q], P], f16).ap()
           for q in range(NWIN)]
    # y table split per int16-gather range so layer-2 range-r gathers only
    # depend on that range's AllGather parts (AG windows align 2:1 with
    # ranges when CALL_CELLS=16: 8*2048=16384 rows per part).
    _rsize = [RBOUND[0], RBOUND[1] - RBOUND[0], RBOUND[2] - RBOUND[1],
              TABROWS - RBOUND[2]]
    y_tr = [nc.dram_tensor(f"y_tr{r}", [_rsize[r], P], f16,
                           addr_space="Shared") for r in range(4)]

    qn = [0]

    def next_q():
        qn[0] = (qn[0] + 1) % 4
        return qn[0]

    with tile.TileContext(nc) as tc:
        with (
            tc.tile_pool(name="meta", bufs=1) as mp,
            tc.tile_pool(name="stage", bufs=3) as sgp,
            tc.tile_pool(name="idxp", bufs=1) as ixp,
            tc.tile_pool(name="selp", bufs=4) as selp,
            tc.tile_pool(name="work", bufs=4) as wp,
            tc.tile_pool(name="ypad", bufs=2) as ypp,
            tc.tile_pool(name="dago", bufs=2) as dgp,
            tc.tile_pool(name="psA", bufs=6, space="PSUM") as ppA,
            tc.tile_pool(name="psB", bufs=2, space="PSUM") as ppB,
        ):
            nc.gpsimd.load_library(library_config.mlp)
            iota_t = mp.tile([P, P], f16, name="iota_t")
            nc.sync.dma_start(iota_t[:], iota_in[:])
            w1_sb = mp.tile([P, P], f16, name="w1_sb")
            nc.sync.dma_start(w1_sb[:], w1t[:])
            wu_sb = mp.tile([P, 4], f16, name="wu_sb")
            nc.sync.dma_start(wu_sb[:], wu_in[:])
            # per-l2-window uv tiles so decode can start before l2 finishes
            uv_w = [mp.tile([P, 4 * CALL_CELLS], f16, name=f"uv_w{wi}")
                    for wi in range(NWIN)]

            def run_layer(key, pr, tabs, mode, after_window=None):
                rel_sb = ixp.tile([P, pr["n_sel"]], f16,
                                  name=f"rel_{key}", tag="relt")
                nc.sync.dma_start(rel_sb[:], rel_in[key][:])
                idx_sb = []
                wst_sb = []
                for r in range(4):
                    cols = pr["layout"][r]["T"] // 16
                    it = ixp.tile([P, cols], i16, name=f"ix_{key}_{r}",
                                  tag=f"ix{r}")
                    nc.sync.dma_start(it[:], idx_in[(key, r)][:])
                    idx_sb.append(it)
                    wt = ixp.tile([P, pr["layout"][r]["T"] // P], f16,
                                  name=f"wst_{key}_{r}", tag=f"wst{r}")
                    nc.sync.dma_start(wt[:], wst_in[(key, r)][:])
                    wst_sb.append(wt)

                for wi in range(pr["nwin"]):
                    k0 = wi * CALL_CELLS
                    k1 = min(k0 + CALL_CELLS, CHUNKS)
                    nk = k1 - k0
                    st_t = {}
                    for r in range(4):
                        call = pr["layout"][r]["calls"][wi]
                        npad = call["n_pad"]
                        nblk = npad // P
                        st = sgp.tile([P, nblk * P], f16,
                                      name=f"st_{key}_{wi}_{r}",
                                      tag=f"stage{r}")
                        nc.gpsimd.dma_gather(
                            st[:].rearrange("p (c e) -> p c e", e=P),
                            tabs[r],
                            idx_sb[r][:, call["base"] // 16:
                                      (call["base"] + npad) // 16],
                            npad, npad, P,
                            queue_num=next_q(), single_packet=False)
                        st_t[r] = st

                    descs = pr["wdescs"][wi]
                    if mode == "T":
                        nbank = (CALL_CELLS + 3) // 4
                        banks = [ppA.tile([P, 4 * P], f32, space="PSUM",
                                          name=f"pt_{key}_{wi}_{t}",
                                          tag="agg")
                                 for t in range(nbank)]

                        def pslice(k):
                            d = k - k0
                            return banks[d // 4][:, (d % 4) * P:
                                                 (d % 4) * P + P]

                        def bank_of(k):
                            return (k - k0) // 4
                    else:
                        wtile = ppA.tile([P, 4 * CALL_CELLS], f32,
                                         space="PSUM",
                                         name=f"pt_{key}_{wi}", tag="agg")

                        def pslice(k):
                            d = k - k0
                            return wtile[:, d * 4:d * 4 + 4]

                        def bank_of(k):
                            return 0

                    first = {}
                    last = {}
                    for j, (r, b, k, s) in enumerate(descs):
                        bk = bank_of(k)
                        first.setdefault(bk, j)
                        last[bk] = j
                    sel_t = {}
                    for j0 in range(0, len(descs), SELBATCH):
                        nb = min(SELBATCH, len(descs) - j0)
                        s0 = descs[j0][3]
                        selt = selp.tile([P, nb * P], f16,
                                         name=f"sel_{key}_{wi}_{j0}",
                                         tag="sel")
                        nc.vector.tensor_tensor(
                            out=selt[:].rearrange("p (b c) -> p b c", c=P),
                            in0=iota_t[:].unsqueeze(1).to_broadcast(
                                [P, nb, P]),
                            in1=rel_sb[:, s0:s0 + nb].unsqueeze(2)
                                .to_broadcast([P, nb, P]),
                            op=eq)
                        for jj in range(nb):
                            sel_t[j0 + jj] = (selt, jj)
                    # weight-scale staged rows after sel builds so the
                    # in-order vector engine does gather-independent sel
                    # work during the gathers' flight time
                    for r in range(4):
                        call = pr["layout"][r]["calls"][wi]
                        nblk = call["n_pad"] // P
                        b0 = call["base"] // P
                        st = st_t[r]
                        if mode == "T":
                            w_exp = (wst_sb[r][:, b0:b0 + nblk].unsqueeze(2)
                                     .to_broadcast([P, nblk, P]))
                            nc.vector.tensor_tensor(
                                out=st[:].rearrange("p (b c) -> p b c", c=P),
                                in0=st[:].rearrange("p (b c) -> p b c", c=P),
                                in1=w_exp, op=mu)
                        else:
                            w_exp = (wst_sb[r][:, b0:b0 + nblk].unsqueeze(2)
                                     .to_broadcast([P, nblk, 4]))
                            v4 = st[:].rearrange(
                                "p (b c) -> p b c", c=P)[:, :, 0:4]
                            nc.vector.tensor_tensor(
                                out=v4, in0=v4, in1=w_exp, op=mu)
                    for j, (r, b, k, s) in enumerate(descs):
                        selt, jj = sel_t[j]
                        bk = bank_of(k)
                        if mode == "T":
                            nc.tensor.matmul(
                                pslice(k),
                                lhsT=st_t[r][:, b * P:(b + 1) * P],
                                rhs=selt[:, jj * P:(jj + 1) * P],
                                start=(first[bk] == j), stop=(last[bk] == j),
                                skip_group_check=True)
                        else:
                            nc.tensor.matmul(
                                pslice(k),
                                lhsT=selt[:, jj * P:(jj + 1) * P],
                                rhs=st_t[r][:, b * P:b * P + 4],
                                start=(first[bk] == j), stop=(last[bk] == j),
                                skip_group_check=True)

                    if mode == "T":
                        # consume: h_T = W1^T agg_T, relu, y = h @ wu
                        yp = ypp.tile([P, nk * P], f16,
                                      name=f"yp_{wi}", tag="yp")
                        nc.vector.memset(yp[:], 0)
                        for k in range(k0, k1):
                            g = k - k0
                            at_sb = wp.tile([P, P], f16, name=f"a1_{k}",
                                            tag="at")
                            nc.scalar.activation(at_sb[:], pslice(k),
                                                 ACT.Copy)
                            ht_ps = ppB.tile([P, P], f32, space="PSUM",
                                             name=f"ht_{k}", tag="psB")
                            nc.tensor.matmul(ht_ps[:], lhsT=w1_sb[:],
                                             rhs=at_sb[:],
                                             start=True, stop=True)
                            ht_sb = wp.tile([P, P], f16, name=f"hts_{k}",
                                            tag="ht")
                            nc.scalar.activation(ht_sb[:], ht_ps[:],
                                                 ACT.Relu)
                            y_ps = ppB.tile([P, 4], f32, space="PSUM",
                                            name=f"y_{k}", tag="psB")
                            nc.tensor.matmul(y_ps[:], lhsT=ht_sb[:],
                                             rhs=wu_sb[:],
                                             start=True, stop=True)
                            nc.scalar.activation(
                                yp[:].rearrange("p (g c) -> p g c",
                                                c=P)[:, g:g + 1, 0:4],
                                y_ps[:].unsqueeze(1), ACT.Copy)
                        nc.sync.dma_start(
                            y_w[wi][:].rearrange("(g p) f -> p g f", p=P),
                            yp[:].rearrange("p (g c) -> p g c", c=P))
                    else:
                        nc.scalar.activation(
                            uv_w[wi][:, 0:nk * 4],
                            wtile[:, 0:nk * 4], ACT.Copy)
                    if after_window is not None:
                        after_window(wi)

            run_layer("l1", l1, [xt[RLO[r]:] for r in range(4)], "T")
            for q in range(NWIN):
                g0 = int(wbase8[q])
                g1 = int(wbase8[q + 1])
                r = int(_range_of(np.array([g0]))[0])
                off = g0 - RLO[r]
                assert g1 <= RLO[r] + _rsize[r], (q, g0, g1, r)
                nc.gpsimd.collective_compute(
                    "AllGather", mybir.AluOpType.bypass,
                    replica_groups=[list(range(NCORES))],
                    ins=[y_w[q][:]],
                    outs=[y_tr[r].ap()[off:off + (g1 - g0), :]])
            # ---------- decode: PE routing out of uv_w, interleaved with l2
            DW = 24  # pair-chunks per psum bank window
            dload = {}
            for name, pr in (("u", du), ("v", dv)):
                fsb = ixp.tile([P, pr["ndesc"]], f16,
                               name=f"df_{name}", tag=f"df_{name}")
                nc.sync.dma_start(fsb[:], dfe_in[(name, 0)][:])
                esb = ixp.tile([P, pr["ndesc"]], f16,
                               name=f"de_{name}", tag=f"de_{name}")
                nc.sync.dma_start(esb[:], dfe_in[(name, 1)][:])
                dload[name] = (fsb, esb)

            def emit_decode_window(name, pr, b0):
                outd = u_out if name == "u" else v_out
                fsb, esb = dload[name]
                descs = pr["descs"]
                b1 = min(b0 + DW, pr["nblk"])
                wd = [(j, d) for j, d in enumerate(descs)
                      if b0 <= d[0] < b1]
                bank = ppA.tile([P, 4 * DW], f32, space="PSUM",
                                name=f"pd_{name}_{b0}", tag="agg")
                sel_t = {}
                for i0 in range(0, len(wd), SELBATCH):
                    nb = min(SELBATCH, len(wd) - i0)
                    s0 = wd[i0][0]
                    t1 = dgp.tile([P, nb * P], f16,
                                  name=f"t1_{name}_{b0}_{i0}", tag="t1")
                    t2 = dgp.tile([P, nb * P], f16,
                                  name=f"t2_{name}_{b0}_{i0}", tag="t2")
                    selt = selp.tile([P, nb * P], f16,
                                     name=f"sd_{name}_{b0}_{i0}", tag="sel")
                    io_rep = iota_t[:].unsqueeze(1).to_broadcast([P, nb, P])
                    nc.vector.tensor_tensor(
                        out=t1[:].rearrange("p (b c) -> p b c", c=P),
                        in0=io_rep,
                        in1=fsb[:, s0:s0 + nb].unsqueeze(2)
                            .to_broadcast([P, nb, P]),
                        op=mybir.AluOpType.is_ge)
                    nc.vector.tensor_tensor(
                        out=t2[:].rearrange("p (b c) -> p b c", c=P),
                        in0=io_rep,
                        in1=esb[:, s0:s0 + nb].unsqueeze(2)
                            .to_broadcast([P, nb, P]),
                        op=mybir.AluOpType.is_ge)
                    nc.vector.tensor_tensor(
                        out=selt[:], in0=t1[:], in1=t2[:],
                        op=mybir.AluOpType.subtract)
                    for jj in range(nb):
                        sel_t[i0 + jj] = (selt, jj)
                first = {}
                last = {}
                for i, (j, (b, k)) in enumerate(wd):
                    first.setdefault(0, i)
                    last[0] = i
                for i, (j, (b, k)) in enumerate(wd):
                    selt, jj = sel_t[i]
                    nc.tensor.matmul(
                        bank[:, (b - b0) * 4:(b - b0) * 4 + 4],
                        lhsT=selt[:, jj * P:(jj + 1) * P],
                        rhs=uv_w[k // CALL_CELLS][
                            :, 4 * (k % CALL_CELLS):4 * (k % CALL_CELLS) + 4],
                        start=(first[0] == i), stop=(last[0] == i),
                        skip_group_check=True)
                dsb = dgp.tile([P, (b1 - b0) * 4], f16,
                               name=f"ds_{name}_{b0}", tag="ds")
                nc.scalar.activation(dsb[:], bank[:, 0:(b1 - b0) * 4],
                                     ACT.Copy)
                nc.sync.dma_start(outd[:, b0 * 4:b1 * 4], dsb[:])

            # decode window -> highest uv-chunk it needs
            dmaxk = {}
            for name, pr in (("u", du), ("v", dv)):
                for b0 in range(0, pr["nblk"], DW):
                    b1 = min(b0 + DW, pr["nblk"])
                    dmaxk[(name, b0)] = max(
                        k for (b, k) in pr["descs"] if b0 <= b < b1)
            dqueue = sorted(dmaxk.items(), key=lambda kv: kv[1])
            dpos = [0]

            def pump_decode(wi):
                kdone = min((wi + 1) * CALL_CELLS, CHUNKS) - 1
                while (dpos[0] < len(dqueue)
                       and dqueue[dpos[0]][1] <= kdone):
                    (nm, b0), _ = dqueue[dpos[0]]
                    emit_decode_window(nm, du if nm == "u" else dv, b0)
                    dpos[0] += 1

            run_layer("l2", l2, [y_tr[r].ap() for r in range(4)], "4",
                      after_window=pump_decode)
            while dpos[0] < len(dqueue):
                (nm, b0), _ = dqueue[dpos[0]]
                emit_decode_window(nm, du if nm == "u" else dv, b0)
                dpos[0] += 1

    nc.compile()

    # ---------- stage inputs & run ----------
    in_maps = []
    for c in range(NCORES):
        m = {"x_tab": x_tab, "W1r": W1.astype(np.float16),
             "wu4": wu, "iota": iota_np,
             "du_first": np.ascontiguousarray(du["dfirst"][c]),
             "du_end": np.ascontiguousarray(du["dend"][c]),
             "dv_first": np.ascontiguousarray(dv["dfirst"][c]),
             "dv_end": np.ascontiguousarray(dv["dend"][c])}
        for key, pr in (("l1", l1), ("l2", l2)):
            m[f"rel_{key}"] = np.ascontiguousarray(pr["rel"][c])
            for r in range(4):
                m[f"idx_{key}_{r}"] = idx_arr[(key, r)][c]
                m[f"wst_{key}_{r}"] = np.ascontiguousarray(pr["wst"][r][c])
        in_maps.append(m)

    res = run_bass_kernel_spmd(nc, in_maps, core_ids=list(range(NCORES)),
                               trace=globals().get("TRACE", False))
    globals()["LAST_EXEC_NS"] = res.exec_time_ns
    globals()["LAST_RES"] = res

    def unpack(key, pr):
        a = np.stack([res.results[c][key] for c in range(NCORES)])
        a = a.reshape(NCORES, P, pr["nblk"], 4).transpose(0, 2, 1, 3)
        return a.reshape(NCORES, pr["T"], 4).astype(np.float32)

    u_res = unpack("u_out", du)
    v_res = unpack("v_out", dv)
    # stream position of each pair within its core's decode stream
    pos0 = np.zeros(npairs, np.int64)
    pos1 = np.zeros(npairs, np.int64)
    for c in range(NCORES):
        pos0[own0 == c] = du["pos"][c]
        pos1[own1 == c] = dv["pos"][c]
    out = u_res[own0, pos0, 0:2] + v_res[own1, pos1, 2:4]
    return np.ascontiguousarray(out, np.float32)
